# revision 39
# baseline (speedup 1.0000x reference)
"""Trainium2 Bass kernel for nn_BiLSTM_7928509628689.

Masked bidirectional LSTM over N=2048 ragged sequences (T=64, D=512, H=256),
returning concat of final fwd/bwd hidden states [N, 2H].

Strategy (8 NeuronCores, data-parallel over N, 256 seqs/core):
  * Sequences globally sorted by length (desc), dealt round-robin to cores,
    right-aligned in time. At step s only the V_s longest sequences are
    active; V_s is baked into the program and every op is trimmed to it.
  * Pad forcing WITHOUT mask matmuls: pad positions of x carry a vector px
    with W_ih[i-rows] @ px = FORCE (solved host-side, underdetermined ->
    exact), so a pad step drives the input gate to sigmoid(FORCE) ~ 0 and,
    with (h,c)=(0,0) inductively, the state stays exactly 0 until the
    sequence starts. No bias/mask rows on the PE at all.
  * Gates^T [4H, V] built in PSUM with slot order g,g,i,i,f,f,o,o so one
    tanh spans slots 0-1 and one sigmoid spans slots 2-7.
  * Multi-step blocks: the x-projection for G consecutive steps (G*Vb <=
    128) is computed by single matmuls of width G*Vb, amortizing
    LDWEIGHTS (the early-step bottleneck: a 128-col weight load takes
    ~53ns while a V-column matmul takes V/2.4ns). Packed blocks alternate
    PSUM halves so block B+1's x-matmuls overlap block B's recurrence.
  * Per step: 16 recurrent matmuls, then one DVE op adds the (per-gate-row)
    bias from a resident tile and copies PSUM->SBUF staging, freeing the
    PSUM banks early; ACT tanh/sigmoid read the staging; DVE does the
    state update on exact active ranges.

kernel(**inputs) takes the FULL unsharded inputs and returns [2048, 512] f32.
"""
import numpy as np

import concourse.tile as tile
from concourse import bacc, mybir
from concourse.bass_utils import run_bass_kernel_spmd
import bass_rust

F32 = mybir.dt.float32
BF16 = mybir.dt.bfloat16
AF = mybir.ActivationFunctionType
OP = mybir.AluOpType

N, T, D, H = 2048, 64, 512, 256
NCORES = 8
NS = N // NCORES           # 256 sequences per core
FH = 4 * H                 # 1024 gate rows
KD = D // 128              # 4 x-projection K chunks
KH = H // 128              # 2 h-projection K chunks
FORCE = -32.0              # i-gate pre-activation for pad steps
DIRS = ("f", "b")
# FH row-chunk order of the 8 psum slots: g,g,i,i,f,f,o,o
SLOT_PERM = [4, 5, 0, 1, 2, 3, 6, 7]

_NC_CACHE = {}


def _inst(r):
    return getattr(r, "ins", r)


def _plan_blocks(V):
    """Greedy multi-step blocks: maximize G with G*Vb <= 128 (Vb = V at the
    block's last step). Returns list of (s0, G, Vb, base); base is the psum
    offset of the block's slot array (packed blocks alternate 0/1024,
    unpacked blocks use base 0 with slot stride 256)."""
    t_steps = len(V)
    blocks = []
    s = 0
    parity = 0
    while s < t_steps:
        G = 1
        for g in range(min(4, t_steps - s), 0, -1):
            if g * int(V[s + g - 1]) <= 128:
                G = g
                break
        Vb = int(V[s + G - 1])
        if G * Vb <= 128:
            base = 1024 * parity
            parity ^= 1
            S = 1 << (G * Vb - 1).bit_length()  # pow2: slots stay bank-aligned
        else:
            base = 0
            S = 256
        blocks.append((s, G, Vb, base, S))
        s += G
    return blocks


def _build(t_steps, V):
    import contextlib

    nc = bacc.Bacc("TRN2", target_bir_lowering=False, debug=False)
    blocks = _plan_blocks(V)
    xsz = [KD * G * Vb for (_, G, Vb, _, _) in blocks]
    xoff = np.concatenate([[0], np.cumsum(xsz)]).astype(int)
    xtot = int(xoff[-1])
    xmax = max(xsz)

    x_dram, wih_d, whh_d, bt_d, bm_d, out_d = {}, {}, {}, {}, {}, {}
    for d in DIRS:
        x_dram[d] = nc.dram_tensor(
            f"x{d}", [128, xtot], BF16, kind="ExternalInput"
        ).ap()
        wih_d[d] = nc.dram_tensor(
            f"wih{d}", [128, KD, FH], BF16, kind="ExternalInput"
        ).ap()
        whh_d[d] = nc.dram_tensor(
            f"whh{d}", [128, KH, FH], BF16, kind="ExternalInput"
        ).ap()
        bt_d[d] = nc.dram_tensor(f"bt{d}", [128, 8, NS], BF16, kind="ExternalInput").ap()
        bm_d[d] = nc.dram_tensor(f"bm{d}", [128, FH], BF16, kind="ExternalInput").ap()
        out_d[d] = nc.dram_tensor(
            f"hT{d}", [128, KH * NS], F32, kind="ExternalOutput"
        ).ap()

    with tile.TileContext(nc) as tc:
        with contextlib.ExitStack() as ctx:
            wpool = ctx.enter_context(tc.tile_pool(name="w", bufs=1))
            xpool = ctx.enter_context(tc.tile_pool(name="x", bufs=3))
            spool = ctx.enter_context(tc.tile_pool(name="state", bufs=1))
            gpool = ctx.enter_context(tc.tile_pool(name="gates", bufs=2))
            opool = ctx.enter_context(tc.tile_pool(name="outs", bufs=1))
            apool = ctx.enter_context(tc.tile_pool(name="acts", bufs=2))
            pspool = ctx.enter_context(tc.tile_pool(name="ps", bufs=1, space="PSUM"))

            wih_t, whh_t, bt_t, bm_t = {}, {}, {}, {}
            for d in DIRS:
                wih_t[d] = wpool.tile([128, KD, FH], BF16, tag=f"wih_{d}", name=f"wih_{d}")
                for k in range(KD):
                    nc.gpsimd.dma_start(
                        wih_t[d][:, k : k + 1, :], wih_d[d][:, k : k + 1, :]
                    )
                whh_t[d] = wpool.tile([128, KH, FH], BF16, tag=f"whh_{d}", name=f"whh_{d}")
                nc.gpsimd.dma_start(whh_t[d][:], whh_d[d][:])
            for d in DIRS:
                # bias tiles last: not needed until the first chain
                bt_t[d] = wpool.tile([128, 8, NS], BF16, tag=f"bt_{d}", name=f"bt_{d}")
                nc.gpsimd.dma_start(bt_t[d][:], bt_d[d][:])
            ones_t = wpool.tile([128, 512], BF16, tag="ones", name="ones")
            nc.vector.memset(ones_t[:], 0.0)
            nc.vector.memset(ones_t[0:1, :], 1.0)

            # persistent state, inactive columns stay zero from this init
            h_t, c_t = {}, {}
            for d in DIRS:
                h_t[d] = spool.tile([128, KH * NS], BF16, tag=f"h_{d}", name=f"h_{d}")
                nc.vector.memset(h_t[d][:], 0.0)
                c_t[d] = spool.tile([128, KH * NS], F32, tag=f"c_{d}", name=f"c_{d}")
                nc.vector.memset(c_t[d][:], 0.0)

            ps_t = {}
            for d in DIRS:
                ps_t[d] = pspool.tile(
                    [128, 4 * 512], F32, tag=f"ps_{d}", name=f"ps_{d}"
                )

            # PE warm-up burst during the initial weight/x DMA window
            wrm = wpool.tile([128, 512], BF16, tag="warm", name="warm")
            nc.vector.memset(wrm[:], 0.0)
            NWARM = 40
            for i in range(NWARM):
                nc.tensor.matmul(
                    ps_t["f"][:, 0:512], wrm[:, 0:128], wrm[:],
                    start=(i == 0), stop=(i == NWARM - 1),
                )

            def bank_of(off):
                return off // 512

            nblocks = len(blocks)
            xts_all = {}

            def emit_dma(bi):
                G, Vb = blocks[bi][1], blocks[bi][2]
                xts_all[bi] = {}
                for d in DIRS:
                    t = xpool.tile([128, xmax], BF16, tag=f"x_{d}", name=f"x_{d}")
                    nc.sync.dma_start(
                        t[:, : xsz[bi]],
                        x_dram[d][:, int(xoff[bi]) : int(xoff[bi]) + xsz[bi]],
                    )
                    xts_all[bi][d] = t

            def emit_pass1(bi, slots, bstate):
                s0, G, Vb, base, S = blocks[bi]
                GV = G * Vb
                for d in DIRS:
                    ps = ps_t[d]
                    xt = xts_all[bi][d]
                    bank_start = bstate.setdefault(d, {})
                    for slot in slots:
                        msl = slice(slot * 128, (slot + 1) * 128)
                        off = base + slot * S
                        o_ap = ps[:, off : off + GV]
                        bk = off // 512
                        for k in range(KD):
                            first = bk not in bank_start and k == 0
                            r = nc.tensor.matmul(
                                o_ap,
                                wih_t[d][:, k, msl],
                                xt[:, k * GV : (k + 1) * GV],
                                start=first, stop=False,
                            )
                            if first:
                                bank_start[bk] = _inst(r)
                            elif k == 0:
                                bass_rust.add_dep_helper(
                                    _inst(r), bank_start[bk], sync=False,
                                    reason="psum bank group order",
                                )

            emit_dma(0)
            emit_pass1(0, range(8), {})
            for bi, (s0, G, Vb, base, S) in enumerate(blocks):
                GV = G * Vb
                packed = GV <= 128
                if bi + 1 < nblocks:
                    emit_dma(bi + 1)
                # prefill: interleave next block's x-projection between this
                # block's steps so the PE queue never stalls behind a
                # chain-dependent h-matmul (legal when both blocks are
                # packed: alternating halves -> disjoint banks, and the
                # half's previous readers finish before our first h-MM)
                prefill = False
                nxt_state = {}
                for j in range(G):
                    s = s0 + j
                    last = s == t_steps - 1
                    for d in DIRS:
                        ps = ps_t[d]
                        for kk in range(KH):
                            for slot in range(8):
                                msl = slice(slot * 128, (slot + 1) * 128)
                                off = base + slot * S + j * Vb
                                bank_end = (
                                    slot == 7
                                    or ((slot + 1) * S) % 512 == 0
                                )
                                nc.tensor.matmul(
                                    ps[:, off : off + Vb],
                                    whh_t[d][:, kk, msl],
                                    h_t[d][:, kk * NS : kk * NS + Vb],
                                    start=False,
                                    stop=(kk == KH - 1 and bank_end),
                                )

                    for d in DIRS:
                        ps = ps_t[d]
                        psv = (
                            ps[:, base : base + 8 * S]
                            .rearrange("p (q n) -> p q n", n=S)[
                                :, :, j * Vb : j * Vb + Vb
                            ]
                        )
                        # bias add + PSUM->SBUF staging copy (frees PSUM)
                        gsb = gpool.tile([128, 8, NS], F32, tag=f"g_{d}", name=f"g_{d}")
                        nc.vector.tensor_tensor(
                            gsb[:, :, :Vb], psv, bt_t[d][:, :, :Vb], OP.add
                        )
                        tg = apool.tile([128, 2, NS], F32, tag=f"tg_{d}", name=f"tg_{d}")
                        nc.scalar.activation(tg[:, :, :Vb], gsb[:, 0:2, :Vb], AF.Tanh)
                        sio = apool.tile([128, 6, NS], F32, tag=f"sio_{d}", name=f"sio_{d}")
                        nc.scalar.activation(sio[:, :, :Vb], gsb[:, 2:8, :Vb], AF.Sigmoid)

                        cc = c_t[d].rearrange("p (q n) -> p q n", n=NS)[:, :, :Vb]
                        hh = h_t[d].rearrange("p (q n) -> p q n", n=NS)[:, :, :Vb]
                        t1 = apool.tile([128, 2, NS], F32, tag=f"t1_{d}", name=f"t1_{d}")
                        nc.vector.tensor_tensor(
                            t1[:, :, :Vb], sio[:, 0:2, :Vb], tg[:, :, :Vb], OP.mult
                        )
                        nc.vector.tensor_tensor(cc, sio[:, 2:4, :Vb], cc, OP.mult)
                        nc.vector.tensor_tensor(cc, cc, t1[:, :, :Vb], OP.add)
                        tcn = apool.tile([128, 2, NS], F32, tag=f"tc_{d}", name=f"tc_{d}")
                        nc.scalar.activation(tcn[:, :, :Vb], cc, AF.Tanh)
                        if last:
                            hf = opool.tile(
                                [128, KH * NS], F32, tag=f"hout_{d}", name=f"hout_{d}"
                            )
                            hfv = hf[:].rearrange("p (q n) -> p q n", n=NS)
                            if Vb < NS:
                                nc.vector.memset(hf[:], 0.0)
                            nc.vector.tensor_tensor(
                                hfv[:, :, :Vb],
                                sio[:, 4:6, :Vb],
                                tcn[:, :, :Vb],
                                OP.mult,
                            )
                            nc.sync.dma_start(out_d[d][:], hf[:])
                        else:
                            nc.vector.tensor_tensor(
                                hh, sio[:, 4:6, :Vb], tcn[:, :, :Vb], OP.mult
                            )
                if bi + 1 < nblocks and not prefill:
                    emit_pass1(bi + 1, range(8), nxt_state)

    nc.compile()
    return nc


def _get_nc(t_steps, V):
    key = (t_steps, tuple(V))
    if key not in _NC_CACHE:
        _NC_CACHE[key] = _build(t_steps, V)
    return _NC_CACHE[key]


def _prep_weights(W_ih, W_hh, b):
    """lhsT layouts (slot-permuted) + bias tile + pad vector px."""
    import ml_dtypes

    wdt = ml_dtypes.bfloat16
    perm = SLOT_PERM
    wih = W_ih.T.reshape(128, KD, 8, 128)[:, :, perm, :].reshape(128, KD, FH)
    wih = np.ascontiguousarray(wih.astype(wdt))
    whh = W_hh.T.reshape(KH, 128, FH).transpose(1, 0, 2)  # (p, kk) <-> 128*kk+p
    whh = whh.reshape(128, KH, 8, 128)[:, :, perm, :].reshape(128, KH, FH)
    whh = np.ascontiguousarray(whh.astype(wdt))
    bp = b.astype(np.float64).reshape(8, 128)[perm]       # [slot, 128]
    btile = np.ascontiguousarray(
        np.broadcast_to(bp.T[:, :, None], (128, 8, NS)).astype(wdt)
    )
    bm = np.zeros((128, FH), np.float32)
    bm[0] = bp.reshape(FH)
    bm = np.ascontiguousarray(bm.astype(wdt))
    # px: W_ih[i-rows] @ px = FORCE (exact; least-norm solution)
    A = W_ih[:H].astype(np.float64)                       # [256, 512]
    rhs = np.full(H, FORCE, np.float64)
    px = A.T @ np.linalg.solve(A @ A.T, rhs)
    return wih, whh, btile, bm, px.astype(np.float32)


def _prep_core(seqs_c, lens_c, t_steps, blocks, px_f, px_b):
    """Per-core x arrays packed per block. seqs_c [NS, T, D] (sorted desc)."""
    import ml_dtypes

    bf16 = ml_dtypes.bfloat16
    ns = seqs_c.shape[0]
    shift = t_steps - lens_c                              # pad steps per seq
    src_t = np.arange(t_steps)[None, :] - shift[:, None]  # [NS, t]
    valid = (src_t >= 0)[..., None]
    gat = seqs_c[np.arange(ns)[:, None], np.clip(src_t, 0, T - 1)]
    xf = np.where(valid, gat, px_f[None, None, :])        # right-aligned
    rev = seqs_c[:, t_steps - 1 :: -1, :]
    xb = np.where(valid, rev, px_b[None, None, :])

    def pack(x_ntd):
        out = []
        for (s0, G, Vb, _, _) in blocks:
            blk = x_ntd[:Vb, s0 : s0 + G, :]              # [Vb, G, D]
            blk = blk.transpose(2, 1, 0).reshape(128, KD, G, Vb)
            out.append(blk.reshape(128, KD * G * Vb))
        return np.ascontiguousarray(np.concatenate(out, axis=1).astype(bf16))

    return pack(xf), pack(xb)


def _unfold(hT):
    """[128, KH*NS] device tile -> [NS, H] h matrix."""
    h_rows = np.concatenate([hT[:, i * NS : (i + 1) * NS] for i in range(KH)], axis=0)
    return h_rows.T  # [NS, H]


def _run(inputs, trace=False, t_cap=None, **spmd_kwargs):
    all_embs = np.asarray(inputs["all_embs"], dtype=np.float32)
    lengths = np.asarray(inputs["lengths"]).astype(np.int64)
    starts = np.asarray(inputs["starts"]).astype(np.int64)

    if np.array_equal(starts, np.arange(N, dtype=np.int64) * T):
        seqs = all_embs.reshape(N, T, D)
    else:
        seqs = all_embs[starts[:, None] + np.arange(T)[None, :]]

    # global sort by length desc, deal round-robin to cores
    order = np.argsort(-lengths, kind="stable")
    t_steps = int(lengths.max())
    if t_cap is not None:
        t_steps = min(t_steps, t_cap)
    core_idx = [order[c::NCORES] for c in range(NCORES)]  # [NCORES][NS]

    # baked active widths: V_s = max over cores of #{len >= t_steps - s}
    Ls = np.stack([np.minimum(lengths[ci], t_steps) for ci in core_idx])
    thr = t_steps - np.arange(t_steps)
    V = (Ls[:, None, :] >= thr[None, :, None]).sum(-1).max(0)
    V = np.maximum(V, 1)

    blocks = _plan_blocks(V)

    w = {}
    for d, (wi, wh, bb) in {
        "f": (inputs["W_ih_f"], inputs["W_hh_f"], inputs["b_f"]),
        "b": (inputs["W_ih_b"], inputs["W_hh_b"], inputs["b_b"]),
    }.items():
        w[d] = _prep_weights(
            np.asarray(wi, np.float32), np.asarray(wh, np.float32),
            np.asarray(bb, np.float32),
        )

    in_maps = []
    for ci in range(NCORES):
        idx = core_idx[ci]
        xf, xb = _prep_core(
            seqs[idx], np.minimum(lengths[idx], t_steps), t_steps, blocks,
            w["f"][4], w["b"][4],
        )
        in_maps.append(
            {
                "xf": xf, "xb": xb,
                "wihf": w["f"][0], "whhf": w["f"][1],
                "btf": w["f"][2], "bmf": w["f"][3],
                "wihb": w["b"][0], "whhb": w["b"][1],
                "btb": w["b"][2], "bmb": w["b"][3],
            }
        )

    nc = _get_nc(t_steps, V)
    res = None
    for attempt in range(3):
        try:
            res = run_bass_kernel_spmd(
                nc, in_maps, core_ids=list(range(NCORES)), trace=trace,
                **spmd_kwargs
            )
            break
        except Exception:
            import traceback as _tb
            _tb.print_exc()
            # rare transient NRT_EXEC_UNIT_UNRECOVERABLE right after a
            # fresh NEFF load; a plain re-execute has always recovered
            if attempt == 2:
                raise
            import time as _time

            _time.sleep(2.0)

    out = np.empty((N, 2 * H), np.float32)
    for ci in range(NCORES):
        out[core_idx[ci], :H] = _unfold(res.results[ci]["hTf"])
        out[core_idx[ci], H:] = _unfold(res.results[ci]["hTb"])
    return out, res


def kernel(**inputs) -> np.ndarray:
    out, _ = _run(inputs)
    return out
